# revision 16
# baseline (speedup 1.0000x reference)
"""Trainium2 Bass kernel for nn_Graph_module_net_0_loss_type_18631568130084.

GNN message-passing block:
  gts       = relu(gt_feat @ Wg + bg)
  attn[i,j] = sigmoid(x[j]@Wq + x[i]@Wk + b_att)          (H == 1)
  atten     = (attn * (mr1+mr2) * col + f_diag) / CHILDS  ([B,H,Nj,Ni])
  o1 = relu(gconv1(x^T)); o1 += ln1(o1 @ atten)^T
  o2 = relu(gconv2(o1));  node_feat = ln2(o2 @ atten);  output2 = (o2 + node_feat^T)^T

Sharding: data-parallel over batch B=16 -> 2 batches per core on 8 cores.

v3 design notes (v1 baseline ~180us, v2 ~135us):
 * Host folds (m1+m2)*score*col into ONE mask msC, with the f_diag term
   folded into the diagonal as f/sigmoid(li+lj); stored fp8 (mask values
   {0,1,2} are exact) and upcast to fp16 by the SWDGE cast-DMA on the
   otherwise-idle gpsimd queue, in parallel with the sync-queue input DMAs.
 * Grouped convs exploit the block-diagonal weight structure (half-width
   matmuls for gconv1, four N=64 matmuls for gconv2).
 * o1_new is transposed j<->m by 2 big DMA xbar transposes per batch.
 * Engine assignment tuned from the v2 trace: all 16 sigmoids issue first
   on ACT (2 table loads total), gconv1 relu on DVE so o1t is ready early,
   gts relu on ACT as low-priority filler, out2 residual adds on gpsimd,
   each conv stage gets its own PSUM pool so a late consumer can't stall
   an unrelated producer's banks.
 * PE order: C0 C1 D0 B0 B1 D1 E0 F0 E1 F1 — gts matmuls act as fillers
   between the big attention contractions.
 * fp16 outputs staged in SBUF, stored in half-batch DMA chunks so the
   store overlaps the layernorm tail.
 * The global 1/CHILDS scale cancels inside both layernorms (eps rescaled).
 * The top-k "col" mask is computed exactly on the host (cheap sufficient
   condition proves col == ones, else exact numpy replica).
"""

import ml_dtypes
import numpy as np

B = 16
N = 1024
CIN = 256
MID = 512
OUT = 256
G = 4
CHILDS = 512
NCORES = 8
B_LOC = B // NCORES  # 2
NT = N // 128  # 8
EPS_LN = 1e-6 * float(CHILDS) ** 2  # eps rescaled because we drop the 1/CHILDS

F16 = np.float16
F32 = np.float32
F8 = ml_dtypes.float8_e4m3

_PROGRAM_CACHE = {}


def _build_program(beta1_nz: bool, beta2_nz: bool):
    import concourse.bacc as bacc
    import concourse.tile as tile
    from concourse import mybir

    f8 = mybir.dt.float8e4
    f16 = mybir.dt.float16
    f32 = mybir.dt.float32
    AF = mybir.ActivationFunctionType
    OP = mybir.AluOpType

    nc = bacc.Bacc("TRN2", debug=False)

    def din(name, shape, dt):
        return nc.dram_tensor(name, shape, dt, kind="ExternalInput").ap()

    def dout(name, shape, dt):
        return nc.dram_tensor(name, shape, dt, kind="ExternalOutput").ap()

    # Per-core inputs (leading dim B_LOC where batch-dependent).
    msC_d = din("msC", [B_LOC, N, N], f8)        # combined mask^T (diag-adjusted)
    xT_d = din("xT", [B_LOC, CIN, N], f16)       # x^T   [c, n]
    gtT_d = din("gtT", [B_LOC, CIN, N], f16)     # gt^T  [c, n]
    lirow_d = din("lirow", [B_LOC, 128, N], f16)  # x@Wk + b_att, host-tiled 128x
    ljT_d = din("ljT", [B_LOC, 128, NT], f32)    # x@Wq chunked      (per-j bias)
    # Replicated weights.
    wg_d = din("wgK", [2, 128, OUT], f16)        # Wg   (c-chunks)
    w1h_d = din("w1h", [2, 128, 256], f16)       # block-diag W1^T halves
    w2g_d = din("w2g", [4, 128, 64], f16)        # block-diag W2^T per group
    bg_d = din("bgrow", [1, OUT], f16)
    b1_d = din("b1row", [1, MID], f16)
    b2_d = din("b2row", [1, OUT], f16)
    g1_d = din("g1row", [128, MID], f16)
    g2_d = din("g2row", [128, OUT], f16)
    beta1_d = din("beta1row", [1, MID], f32)
    beta2_d = din("beta2row", [1, OUT], f32)
    ones_d = din("onescol", [1, 128], f16)

    gts_d = dout("gts", [B_LOC, N, OUT], f16)
    node_d = dout("node", [B_LOC, N, OUT], f16)
    out2_d = dout("out2", [B_LOC, N, OUT], f16)

    with tile.TileContext(nc) as tc:
        with tc.tile_pool(name="const", bufs=1) as constp, \
             tc.tile_pool(name="big", bufs=1) as bigp, \
             tc.tile_pool(name="work", bufs=4) as workp, \
             tc.tile_pool(name="mm", bufs=4, space="PSUM") as mmp, \
             tc.tile_pool(name="cp", bufs=2, space="PSUM") as cpp, \
             tc.tile_pool(name="bp", bufs=2, space="PSUM") as bpp:

            # ---- tiny per-batch rows FIRST: they gate all 16 sigmoids.
            # Two-hop broadcast: HBM -> [1,N] (1 descriptor) then SBUF->SBUF
            # fan-out (no HBM small-descriptor penalty).
            lirow_t = {}
            ljT_t = {}
            for b in range(B_LOC):
                lirow_t[b] = bigp.tile([128, N], f16, tag=f"lirow{b}", name=f"lirowt{b}")
                ljT_t[b] = bigp.tile([128, NT], f32, tag=f"ljT{b}", name=f"ljTt{b}")
                nc.sync.dma_start(out=ljT_t[b], in_=ljT_d[b])
                nc.sync.dma_start(out=lirow_t[b], in_=lirow_d[b])

            # ---- constants ----
            ones_t = constp.tile([1, 128], f16)
            nc.sync.dma_start(out=ones_t, in_=ones_d)
            wg_t = constp.tile([128, 2, OUT], f16)
            nc.sync.dma_start(out=wg_t, in_=wg_d.rearrange("c p f -> p c f"))
            w1h_t = constp.tile([128, 2, 256], f16)
            nc.sync.dma_start(out=w1h_t, in_=w1h_d.rearrange("c p f -> p c f"))
            w2g_t = constp.tile([128, 4, 64], f16)
            nc.sync.dma_start(out=w2g_t, in_=w2g_d.rearrange("c p f -> p c f"))
            bg_t = constp.tile([1, OUT], f16)
            nc.sync.dma_start(out=bg_t, in_=bg_d)
            b1_t = constp.tile([1, MID], f16)
            nc.sync.dma_start(out=b1_t, in_=b1_d)
            b2_t = constp.tile([1, OUT], f16)
            nc.sync.dma_start(out=b2_t, in_=b2_d)
            g1row_t = constp.tile([128, MID], f16)
            nc.sync.dma_start(out=g1row_t, in_=g1_d)
            g2row_t = constp.tile([128, OUT], f16)
            nc.sync.dma_start(out=g2row_t, in_=g2_d)
            if beta1_nz:
                beta1_t = constp.tile([128, MID], f32)
                nc.sync.dma_start(out=beta1_t, in_=beta1_d.to_broadcast([128, MID]))
            if beta2_nz:
                beta2_t = constp.tile([128, OUT], f32)
                nc.sync.dma_start(out=beta2_t, in_=beta2_d.to_broadcast([128, OUT]))
            eps_t = constp.tile([128, 1], f32)
            nc.vector.memset(eps_t, EPS_LN)

            # ---- per-batch persistent tiles ----
            At = {}
            msb = {}
            o1t = {}
            o1nT = {}
            o1n = {}
            o2t = {}
            gts_s = {}
            node_s = {}
            out2_s = {}
            xT_t = {}
            gtT_t = {}
            for b in range(B_LOC):
                At[b] = bigp.tile([128, NT, N], f16, tag=f"At{b}", name=f"At{b}")
                msb[b] = bigp.tile([128, NT, N], f16, tag=f"ms{b}", name=f"ms{b}")
                o1t[b] = bigp.tile([128, NT, MID], f16, tag=f"o1t{b}", name=f"o1t{b}")
                o1nT[b] = bigp.tile([128, NT, MID], f16, tag=f"o1nT{b}", name=f"o1nT{b}")
                o1n[b] = bigp.tile([128, NT, 4, 128], f16, tag=f"o1n{b}", name=f"o1n{b}")
                o2t[b] = bigp.tile([128, NT, OUT], f16, tag=f"o2t{b}", name=f"o2t{b}")
                gts_s[b] = bigp.tile([128, NT, OUT], f16, tag=f"gts{b}", name=f"gtss{b}")
                node_s[b] = bigp.tile([128, NT, OUT], f16, tag=f"node{b}", name=f"nodes{b}")
                out2_s[b] = bigp.tile([128, NT, OUT], f16, tag=f"out2{b}", name=f"out2s{b}")
                xT_t[b] = bigp.tile([128, 2, N], f16, tag=f"xT{b}", name=f"xTt{b}")
                gtT_t[b] = bigp.tile([128, 2, N], f16, tag=f"gtT{b}", name=f"gtTt{b}")

            # ---- input DMAs ----
            for b in range(B_LOC):
                nc.sync.dma_start(
                    out=xT_t[b], in_=xT_d[b].rearrange("(c p) n -> p c n", p=128)
                )
            for b in range(B_LOC):
                nc.sync.dma_start(
                    out=gtT_t[b], in_=gtT_d[b].rearrange("(c p) n -> p c n", p=128)
                )
            # gpsimd queue (parallel to sync): fp8 masks, cast to fp16 in-flight
            for b in range(B_LOC):
                for h in range(2):
                    nc.gpsimd.dma_start(
                        out=msb[b][:, 4 * h : 4 * h + 4, :],
                        in_=msC_d[b, 512 * h : 512 * (h + 1), :].rearrange(
                            "(t p) i -> p t i", p=128
                        ),
                    )

            # ---- Phase 1 ----
            # Per-tile interleave of A (sigmoid on ACT, mask-mult on DVE) and
            # C (gconv1 matmuls on PE, relu on DVE) so the DVE queue drains
            # each batch's At and o1t tiles in lockstep with the sigmoids.
            def stage_AC_D0h1(b, ps_h1):
                # Per-jc: sigmoid (ACT), mask-mult (DVE), gconv1 (PE), relu
                # (DVE), then the first 4 it-groups of the big contraction so
                # the PE rides the sigmoid window. The 4 PSUM groups stay open
                # across the whole jc sweep.
                for jc in range(NT):
                    nc.scalar.activation(
                        out=At[b][:, jc, :], in_=lirow_t[b], func=AF.Sigmoid,
                        bias=ljT_t[b][:, jc : jc + 1], scale=1.0,
                    )
                    nc.vector.tensor_mul(
                        At[b][:, jc, :], At[b][:, jc, :], msb[b][:, jc, :]
                    )
                    ps = cpp.tile([128, MID], mybir.dt.float32, tag="cps")
                    nc.tensor.matmul(ps, lhsT=ones_t, rhs=b1_t, start=True, stop=False)
                    for cc in range(2):
                        nc.tensor.matmul(
                            ps[:, 256 * cc : 256 * (cc + 1)],
                            lhsT=xT_t[b][:, cc, jc * 128 : (jc + 1) * 128],
                            rhs=w1h_t[:, cc, :],
                            start=False, stop=(cc == 1),
                            skip_group_check=True,
                        )
                    # relu on DVE so o1t is ready independent of the ACT queue
                    nc.vector.tensor_scalar_max(o1t[b][:, jc, :], ps, 0.0)

            def stage_B(b):
                for nt in range(NT):
                    ps = bpp.tile([128, OUT], mybir.dt.float32, tag="bps", name="bps")
                    nc.tensor.matmul(ps, lhsT=ones_t, rhs=bg_t, start=True, stop=False)
                    for cc in range(2):
                        nc.tensor.matmul(
                            ps,
                            lhsT=gtT_t[b][:, cc, nt * 128 : (nt + 1) * 128],
                            rhs=wg_t[:, cc, :],
                            start=False, stop=(cc == 1),
                        )
                    nc.scalar.activation(
                        out=gts_s[b][:, nt, :], in_=ps, func=AF.Relu
                    )
                nc.scalar.dma_start(
                    out=gts_d[b].rearrange("(t p) f -> p t f", p=128), in_=gts_s[b]
                )

            def ln1_apply(b, it, ps):
                    sv = workp.tile([128, 6], f32, tag="sv")
                    nc.vector.bn_stats(out=sv, in_=ps)
                    mv = workp.tile([128, 2], f32, tag="mv")
                    nc.vector.bn_aggr(out=mv, in_=sv)
                    std = workp.tile([128, 1], f32, tag="std")
                    nc.scalar.activation(
                        out=std, in_=mv[:, 1:2], func=AF.Sqrt, bias=eps_t
                    )
                    rstd = workp.tile([128, 1], f32, tag="rstd")
                    nc.vector.reciprocal(out=rstd, in_=std)
                    outer = workp.tile([128, MID], f16, tag="outer")
                    nc.vector.tensor_scalar_mul(outer, g1row_t, rstd)
                    ln = workp.tile([128, MID], f16, tag="ln")
                    nc.vector.scalar_tensor_tensor(
                        out=ln, in0=ps, scalar=mv[:, 0:1], in1=outer,
                        op0=OP.subtract, op1=OP.mult,
                    )
                    if beta1_nz:
                        nc.vector.tensor_add(ln, ln, beta1_t)
                    nc.vector.tensor_add(o1nT[b][:, it, :], ln, o1t[b][:, it, :])

            def stage_D0_rest(b, ps_h1):
                for it in range(4):
                    ln1_apply(b, it, ps_h1[it])
                for it in range(4, NT):
                    ps = mmp.tile([128, MID], mybir.dt.float32, tag="ps")
                    for jc in range(NT):
                        nc.tensor.matmul(
                            ps,
                            lhsT=At[b][:, jc, it * 128 : (it + 1) * 128],
                            rhs=o1t[b][:, jc, :],
                            start=(jc == 0), stop=(jc == NT - 1),
                        )
                    ln1_apply(b, it, ps)

            def stage_D(b):
                for it in range(NT):
                    ps = mmp.tile([128, MID], mybir.dt.float32, tag="ps")
                    for jc in range(NT):
                        nc.tensor.matmul(
                            ps,
                            lhsT=At[b][:, jc, it * 128 : (it + 1) * 128],
                            rhs=o1t[b][:, jc, :],
                            start=(jc == 0), stop=(jc == NT - 1),
                        )
                    ln1_apply(b, it, ps)

            def stage_T(b):
                # DMA xbar transpose, 4 it-tiles at a time: [128, 2048] ->
                # o1n[128, (it,mc), 128] with row = mc*128 + mp.
                for q in range(2):
                    nc.sync.dma_start(
                        out=o1n[b][:, 4 * q : 4 * q + 4, :, :],
                        in_=o1nT[b][:, 4 * q : 4 * q + 4, :],
                        transpose=True,
                    )

            def stage_E(b):
                for jt in range(NT):
                    ps = mmp.tile([128, MID], mybir.dt.float32, tag="ps")
                    p256 = ps[:, :OUT]
                    nc.tensor.matmul(p256, lhsT=ones_t, rhs=b2_t, start=True, stop=False)
                    for g in range(4):
                        nc.tensor.matmul(
                            p256[:, 64 * g : 64 * (g + 1)],
                            lhsT=o1n[b][:, jt, g, :],
                            rhs=w2g_t[:, g, :],
                            start=False, stop=(g == 3),
                            skip_group_check=True,
                        )
                    nc.scalar.activation(out=o2t[b][:, jt, :], in_=p256, func=AF.Relu)

            def stage_F(b):
                for it in range(NT):
                    ps = mmp.tile([128, MID], mybir.dt.float32, tag="ps")
                    p256 = ps[:, :OUT]
                    for jc in range(NT):
                        nc.tensor.matmul(
                            p256,
                            lhsT=At[b][:, jc, it * 128 : (it + 1) * 128],
                            rhs=o2t[b][:, jc, :],
                            start=(jc == 0), stop=(jc == NT - 1),
                        )
                    sv = workp.tile([128, 6], f32, tag="sv")
                    nc.vector.bn_stats(out=sv, in_=p256)
                    mv = workp.tile([128, 2], f32, tag="mv")
                    nc.vector.bn_aggr(out=mv, in_=sv)
                    std = workp.tile([128, 1], f32, tag="std")
                    nc.scalar.activation(
                        out=std, in_=mv[:, 1:2], func=AF.Sqrt, bias=eps_t
                    )
                    rstd = workp.tile([128, 1], f32, tag="rstd")
                    nc.vector.reciprocal(out=rstd, in_=std)
                    outer2 = workp.tile([128, OUT], f16, tag="outer2")
                    nc.vector.tensor_scalar_mul(outer2, g2row_t, rstd)
                    nc.vector.scalar_tensor_tensor(
                        out=node_s[b][:, it, :], in0=p256, scalar=mv[:, 0:1],
                        in1=outer2, op0=OP.subtract, op1=OP.mult,
                    )
                    if beta2_nz:
                        nc.vector.tensor_add(
                            node_s[b][:, it, :], node_s[b][:, it, :], beta2_t
                        )
                    # residual add on gpsimd (idle engine) to shorten the DVE tail
                    nc.gpsimd.tensor_add(
                        out2_s[b][:, it, :], node_s[b][:, it, :], o2t[b][:, it, :]
                    )
                    if it == 3 or it == 7:
                        h = it // 4
                        nc.scalar.dma_start(
                            out=node_d[b].rearrange("(t p) f -> p t f", p=128)[
                                :, 4 * h : 4 * h + 4, :
                            ],
                            in_=node_s[b][:, 4 * h : 4 * h + 4, :],
                        )
                        nc.scalar.dma_start(
                            out=out2_d[b].rearrange("(t p) f -> p t f", p=128)[
                                :, 4 * h : 4 * h + 4, :
                            ],
                            in_=out2_s[b][:, 4 * h : 4 * h + 4, :],
                        )

            def stage_sig(b):
                for jt in range(NT):
                    nc.scalar.activation(
                        out=At[b][:, jt, :], in_=lirow_t[b], func=AF.Sigmoid,
                        bias=ljT_t[b][:, jt : jt + 1], scale=1.0,
                    )

            def stage_Atm_gpsimd(b):
                # mask-mult on the gpsimd engine (frees the DVE queue; the
                # sigmoids pace this anyway)
                for jt in range(NT):
                    nc.gpsimd.tensor_mul(
                        At[b][:, jt, :], At[b][:, jt, :], msb[b][:, jt, :]
                    )

            def stage_Conly(b):
                for jt in range(NT):
                    ps = cpp.tile([128, MID], mybir.dt.float32, tag="cps")
                    nc.tensor.matmul(ps, lhsT=ones_t, rhs=b1_t, start=True, stop=False)
                    for cc in range(2):
                        nc.tensor.matmul(
                            ps[:, 256 * cc : 256 * (cc + 1)],
                            lhsT=xT_t[b][:, cc, jt * 128 : (jt + 1) * 128],
                            rhs=w1h_t[:, cc, :],
                            start=False, stop=(cc == 1),
                            skip_group_check=True,
                        )
                    nc.vector.tensor_scalar_max(o1t[b][:, jt, :], ps, 0.0)

            # Emission order tuned against in-order engine queues:
            # PE: C0 D0 C1 B0 B1 D1 E0 F0 E1 F1; ACT: sig0 sig1 then LN ops;
            # DVE: b0 A/C pairs, D0-LN, C1 relus, D1-LN, F-LN chains;
            # gpsimd: mask DMAs, batch-1 At mults, out2 adds.
            stage_AC_D0h1(0, None)
            stage_AC_D0h1(1, None)
            stage_D(0)
            stage_T(0)
            stage_B(0)
            stage_B(1)
            stage_D(1)
            stage_T(1)
            stage_E(0)
            stage_F(0)
            stage_E(1)
            stage_F(1)

    nc.compile()
    return nc


def _compute_col_fast(m1, m2, sm):
    """Exact col == ones proof via a cheap sufficient condition, else None."""
    if m1.min() < 0.0 or m2.min() < 0.0 or sm.min() < 0.0:
        return None
    spos = (sm > 0).astype(F32)
    colnz = np.zeros(N, dtype=bool)
    nz1max = 0.0
    nz2max = 0.0
    for b in range(B):
        p1 = (m1[b] > 0).astype(F32)
        p2 = (m2[b] > 0).astype(F32)
        nz1max = max(nz1max, float((p1 @ spos[b]).max()))
        nz2max = max(nz2max, float((p2 @ spos[b]).max()))
        colnz |= ((p1 + p2).max(axis=0) > 0) & (spos[b] > 0)
    if nz1max <= CHILDS // 4 and nz2max <= CHILDS // 2 and colnz.all():
        return np.ones(N, dtype=F32)
    return None


def _compute_col_slow(m1, m2, sm, li, lj):
    """Exact replica of the reference top-k column-union (numpy)."""
    k4, k2 = CHILDS // 4, CHILDS // 2
    col = np.zeros(N, dtype=bool)
    for b in range(B):
        logits = li[b][:, None] + lj[b][None, :]
        a = 1.0 / (1.0 + np.exp(-logits.astype(F32)))
        mr1 = m1[b] * sm[b][None, :]
        mr2 = m2[b] * sm[b][None, :]
        a1 = a * mr1
        a2 = a * mr2
        # lax.top_k ties -> lowest index; stable argsort on (-a) reproduces it.
        col[np.argsort(-a1, axis=1, kind="stable")[:, :k4].ravel()] = True
        col[np.argsort(a1, axis=1, kind="stable")[:, :k4].ravel()] = True
        col[np.argsort(-a2, axis=1, kind="stable")[:, :k2].ravel()] = True
        col[np.argsort(a2, axis=1, kind="stable")[:, :k4].ravel()] = True
    return col.astype(F32)


def kernel(**inputs):
    x = np.ascontiguousarray(np.asarray(inputs["x"], dtype=F32))
    m1 = np.asarray(inputs["masks_roi1"], dtype=F32)
    m2 = np.asarray(inputs["masks_roi2"], dtype=F32)
    sm = np.asarray(inputs["score_mask"], dtype=F32)
    gt = np.asarray(inputs["gt_feat"], dtype=F32)
    W_att = np.asarray(inputs["W_att"], dtype=F32)
    b_att = np.asarray(inputs["b_att"], dtype=F32)
    W1 = np.asarray(inputs["W1"], dtype=F32)
    b1 = np.asarray(inputs["b1"], dtype=F32)
    W2 = np.asarray(inputs["W2"], dtype=F32)
    b2 = np.asarray(inputs["b2"], dtype=F32)
    g1 = np.asarray(inputs["g1"], dtype=F32)
    beta1 = np.asarray(inputs["beta1"], dtype=F32)
    g2 = np.asarray(inputs["g2"], dtype=F32)
    beta2 = np.asarray(inputs["beta2"], dtype=F32)
    Wg = np.asarray(inputs["Wg"], dtype=F32)
    bg = np.asarray(inputs["bg"], dtype=F32)

    assert x.shape == (B, N, CIN) and W_att.shape == (2 * CIN, 1)

    # ---- host prep: tiny vector math + layout/dtype staging ----
    lj = x.reshape(B * N, CIN) @ W_att[:CIN, 0]
    lj = lj.reshape(B, N)
    li = x.reshape(B * N, CIN) @ W_att[CIN:, 0]
    li = li.reshape(B, N) + b_att[0]

    col = _compute_col_fast(m1, m2, sm)
    if col is None:
        col = _compute_col_slow(m1, m2, sm, li, lj)

    # Combined transposed mask: msC[b,j,i] = (m1[b,i,j]+m2[b,i,j])*sm[b,j]*col[j]
    # with the f_diag term folded into the diagonal as f/sigmoid(li+lj) so the
    # device-side multiply by sigmoid reconstructs it.  Mask values {0,1,2} are
    # exact in fp8; the diagonal entries round at ~6% relative, which after the
    # (1/CHILDS-scaled) contraction is noise well below the fp16 level.
    smcol = sm * col[None, :]
    msC = (m1.transpose(0, 2, 1) + m2.transpose(0, 2, 1)) * smcol[:, :, None]
    f = (sm == 0).astype(F32)
    diag_sig = 1.0 / (1.0 + np.exp(-(li + lj).astype(np.float64)))
    idx = np.arange(N)
    msC[:, idx, idx] += f / diag_sig.astype(F32)
    msC = msC.astype(F8)

    xT = np.ascontiguousarray(x.transpose(0, 2, 1)).astype(F16)
    gtT = np.ascontiguousarray(gt.transpose(0, 2, 1)).astype(F16)
    lirow = np.broadcast_to(li.astype(F16)[:, None, :], (B, 128, N)).copy()
    ljT = np.ascontiguousarray(lj.reshape(B, NT, 128).transpose(0, 2, 1)).astype(F32)

    # Weights: block-diagonal transposed layouts for the grouped convs.
    w1bd = np.zeros((CIN, MID), dtype=F32)
    for g in range(G):
        w1bd[64 * g : 64 * (g + 1), 128 * g : 128 * (g + 1)] = W1[
            128 * g : 128 * (g + 1), :
        ].T
    w1h = np.stack(
        [w1bd[0:128, 0:256], w1bd[128:256, 256:512]]
    ).astype(F16)  # [2, 128, 256]
    w2g = np.stack(
        [W2[64 * g : 64 * (g + 1), :].T for g in range(G)]
    ).astype(F16)  # [4, 128, 64]
    wgK = np.ascontiguousarray(Wg.reshape(2, 128, OUT)).astype(F16)

    shared = {
        "wgK": wgK,
        "w1h": w1h,
        "w2g": w2g,
        "bgrow": bg.reshape(1, OUT).astype(F16),
        "b1row": b1.reshape(1, MID).astype(F16),
        "b2row": b2.reshape(1, OUT).astype(F16),
        "g1row": np.broadcast_to(g1.astype(F16)[None, :], (128, MID)).copy(),
        "g2row": np.broadcast_to(g2.astype(F16)[None, :], (128, OUT)).copy(),
        "beta1row": beta1.reshape(1, MID).astype(F32),
        "beta2row": beta2.reshape(1, OUT).astype(F32),
        "onescol": np.ones((1, 128), dtype=F16),
    }
    in_maps = []
    for c in range(NCORES):
        s = slice(B_LOC * c, B_LOC * (c + 1))
        in_maps.append(
            {
                "msC": msC[s],
                "xT": xT[s],
                "gtT": gtT[s],
                "lirow": lirow[s],
                "ljT": ljT[s],
                **shared,
            }
        )

    beta_key = (bool(np.any(beta1)), bool(np.any(beta2)))
    if beta_key not in _PROGRAM_CACHE:
        _PROGRAM_CACHE[beta_key] = _build_program(*beta_key)
    nc = _PROGRAM_CACHE[beta_key]

    global _LAST_IN_MAPS
    _LAST_IN_MAPS = in_maps

    from concourse.bass_utils import run_bass_kernel_spmd

    res = run_bass_kernel_spmd(nc, in_maps, core_ids=list(range(NCORES)))
    results = res.results if hasattr(res, "results") else res

    output2 = np.concatenate([results[c]["out2"] for c in range(NCORES)], axis=0)
    gts = np.concatenate([results[c]["gts"] for c in range(NCORES)], axis=0)
    node_feat = np.concatenate([results[c]["node"] for c in range(NCORES)], axis=0)
    return output2.astype(F32), gts.astype(F32), node_feat.astype(F32)


# revision 18
# speedup vs baseline: 1.0344x; 1.0344x over previous
"""Trainium2 Bass kernel for nn_Graph_module_net_0_loss_type_18631568130084.

GNN message-passing block:
  gts       = relu(gt_feat @ Wg + bg)
  attn[i,j] = sigmoid(x[j]@Wq + x[i]@Wk + b_att)          (H == 1)
  atten     = (attn * (mr1+mr2) * col + f_diag) / CHILDS  ([B,H,Nj,Ni])
  o1 = relu(gconv1(x^T)); o1 += ln1(o1 @ atten)^T
  o2 = relu(gconv2(o1));  node_feat = ln2(o2 @ atten);  output2 = (o2 + node_feat^T)^T

Sharding: data-parallel over batch B=16 -> 2 batches per core on 8 cores.

v3 design notes (v1 baseline ~180us, v2 ~135us):
 * Host folds (m1+m2)*score*col into ONE mask msC, with the f_diag term
   folded into the diagonal as f/sigmoid(li+lj); stored fp8 (mask values
   {0,1,2} are exact) and upcast to fp16 by the SWDGE cast-DMA on the
   otherwise-idle gpsimd queue, in parallel with the sync-queue input DMAs.
 * Grouped convs exploit the block-diagonal weight structure (half-width
   matmuls for gconv1, four N=64 matmuls for gconv2).
 * o1_new is transposed j<->m by 2 big DMA xbar transposes per batch.
 * Engine assignment tuned from the v2 trace: all 16 sigmoids issue first
   on ACT (2 table loads total), gconv1 relu on DVE so o1t is ready early,
   gts relu on ACT as low-priority filler, out2 residual adds on gpsimd,
   each conv stage gets its own PSUM pool so a late consumer can't stall
   an unrelated producer's banks.
 * PE order: C0 C1 D0 B0 B1 D1 E0 F0 E1 F1 — gts matmuls act as fillers
   between the big attention contractions.
 * fp16 outputs staged in SBUF, stored in half-batch DMA chunks so the
   store overlaps the layernorm tail.
 * The global 1/CHILDS scale cancels inside both layernorms (eps rescaled).
 * The top-k "col" mask is computed exactly on the host (cheap sufficient
   condition proves col == ones, else exact numpy replica).
"""

import ml_dtypes
import numpy as np

B = 16
N = 1024
CIN = 256
MID = 512
OUT = 256
G = 4
CHILDS = 512
NCORES = 8
B_LOC = B // NCORES  # 2
NT = N // 128  # 8
EPS_LN = 1e-6 * float(CHILDS) ** 2  # eps rescaled because we drop the 1/CHILDS

F16 = np.float16
F32 = np.float32
F8 = ml_dtypes.float8_e4m3

_PROGRAM_CACHE = {}


def _build_program(beta1_nz: bool, beta2_nz: bool):
    import concourse.bacc as bacc
    import concourse.tile as tile
    from concourse import mybir

    f8 = mybir.dt.float8e4
    f16 = mybir.dt.float16
    f32 = mybir.dt.float32
    AF = mybir.ActivationFunctionType
    OP = mybir.AluOpType

    nc = bacc.Bacc("TRN2", debug=False)

    def din(name, shape, dt):
        return nc.dram_tensor(name, shape, dt, kind="ExternalInput").ap()

    def dout(name, shape, dt):
        return nc.dram_tensor(name, shape, dt, kind="ExternalOutput").ap()

    # Per-core inputs (leading dim B_LOC where batch-dependent).
    msC_d = din("msC", [B_LOC, N, N], f8)        # combined mask^T (diag-adjusted)
    xT_d = din("xT", [B_LOC, CIN, N], f16)       # x^T   [c, n]
    gtT_d = din("gtT", [B_LOC, CIN, N], f16)     # gt^T  [c, n]
    lirow_d = din("lirow", [B_LOC, 128, N], f16)  # x@Wk + b_att, host-tiled 128x
    ljT_d = din("ljT", [B_LOC, 128, NT], f32)    # x@Wq chunked      (per-j bias)
    # Replicated weights.
    wg_d = din("wgK", [2, 128, OUT], f16)        # Wg   (c-chunks)
    w1h_d = din("w1h", [2, 128, 256], f16)       # block-diag W1^T halves
    w2g_d = din("w2g", [4, 128, 64], f16)        # block-diag W2^T per group
    bg_d = din("bgrow", [1, OUT], f16)
    b1_d = din("b1row", [1, MID], f16)
    b2_d = din("b2row", [1, OUT], f16)
    g1_d = din("g1row", [128, MID], f16)
    g2_d = din("g2row", [128, OUT], f16)
    beta1_d = din("beta1row", [1, MID], f32)
    beta2_d = din("beta2row", [1, OUT], f32)
    ones_d = din("onescol", [1, 128], f16)

    gts_d = dout("gts", [B_LOC, N, OUT], f16)
    node_d = dout("node", [B_LOC, N, OUT], f16)
    out2_d = dout("out2", [B_LOC, N, OUT], f16)

    with tile.TileContext(nc) as tc:
        with tc.tile_pool(name="const", bufs=1) as constp, \
             tc.tile_pool(name="big", bufs=1) as bigp, \
             tc.tile_pool(name="work", bufs=4) as workp, \
             tc.tile_pool(name="mm", bufs=4, space="PSUM") as mmp, \
             tc.tile_pool(name="cp", bufs=2, space="PSUM") as cpp, \
             tc.tile_pool(name="bp", bufs=2, space="PSUM") as bpp:

            # ---- tiny per-batch rows FIRST: they gate all 16 sigmoids.
            # Two-hop broadcast: HBM -> [1,N] (1 descriptor) then SBUF->SBUF
            # fan-out (no HBM small-descriptor penalty).
            lirow_t = {}
            ljT_t = {}
            for b in range(B_LOC):
                lirow_t[b] = bigp.tile([128, N], f16, tag=f"lirow{b}", name=f"lirowt{b}")
                ljT_t[b] = bigp.tile([128, NT], f32, tag=f"ljT{b}", name=f"ljTt{b}")
                nc.sync.dma_start(out=ljT_t[b], in_=ljT_d[b])
                nc.sync.dma_start(out=lirow_t[b], in_=lirow_d[b])

            # ---- constants ----
            ones_t = constp.tile([1, 128], f16)
            nc.sync.dma_start(out=ones_t, in_=ones_d)
            wg_t = constp.tile([128, 2, OUT], f16)
            nc.sync.dma_start(out=wg_t, in_=wg_d.rearrange("c p f -> p c f"))
            w1h_t = constp.tile([128, 2, 256], f16)
            nc.sync.dma_start(out=w1h_t, in_=w1h_d.rearrange("c p f -> p c f"))
            w2g_t = constp.tile([128, 4, 64], f16)
            nc.sync.dma_start(out=w2g_t, in_=w2g_d.rearrange("c p f -> p c f"))
            bg_t = constp.tile([1, OUT], f16)
            nc.sync.dma_start(out=bg_t, in_=bg_d)
            b1_t = constp.tile([1, MID], f16)
            nc.sync.dma_start(out=b1_t, in_=b1_d)
            b2_t = constp.tile([1, OUT], f16)
            nc.sync.dma_start(out=b2_t, in_=b2_d)
            g1row_t = constp.tile([128, MID], f16)
            nc.sync.dma_start(out=g1row_t, in_=g1_d)
            g2row_t = constp.tile([128, OUT], f16)
            nc.sync.dma_start(out=g2row_t, in_=g2_d)
            if beta1_nz:
                beta1_t = constp.tile([128, MID], f32)
                nc.sync.dma_start(out=beta1_t, in_=beta1_d.to_broadcast([128, MID]))
            if beta2_nz:
                beta2_t = constp.tile([128, OUT], f32)
                nc.sync.dma_start(out=beta2_t, in_=beta2_d.to_broadcast([128, OUT]))
            eps_t = constp.tile([128, 1], f32)
            nc.vector.memset(eps_t, EPS_LN)

            # ---- per-batch persistent tiles ----
            At = {}
            msb = {}
            o1t = {}
            o1nT = {}
            o1n = {}
            o2t = {}
            gts_s = {}
            node_s = {}
            out2_s = {}
            xT_t = {}
            gtT_t = {}
            for b in range(B_LOC):
                At[b] = bigp.tile([128, NT, N], f16, tag=f"At{b}", name=f"At{b}")
                msb[b] = bigp.tile([128, NT, N], f16, tag=f"ms{b}", name=f"ms{b}")
                o1t[b] = bigp.tile([128, NT, MID], f16, tag=f"o1t{b}", name=f"o1t{b}")
                o1nT[b] = bigp.tile([128, NT, MID], f16, tag=f"o1nT{b}", name=f"o1nT{b}")
                o1n[b] = bigp.tile([128, NT, 4, 128], f16, tag=f"o1n{b}", name=f"o1n{b}")
                o2t[b] = bigp.tile([128, NT, OUT], f16, tag=f"o2t{b}", name=f"o2t{b}")
                gts_s[b] = bigp.tile([128, NT, OUT], f16, tag=f"gts{b}", name=f"gtss{b}")
                node_s[b] = bigp.tile([128, NT, OUT], f16, tag=f"node{b}", name=f"nodes{b}")
                out2_s[b] = bigp.tile([128, NT, OUT], f16, tag=f"out2{b}", name=f"out2s{b}")
                xT_t[b] = bigp.tile([128, 2, N], f16, tag=f"xT{b}", name=f"xTt{b}")
                gtT_t[b] = bigp.tile([128, 2, N], f16, tag=f"gtT{b}", name=f"gtTt{b}")

            # ---- input DMAs ----
            for b in range(B_LOC):
                nc.sync.dma_start(
                    out=xT_t[b], in_=xT_d[b].rearrange("(c p) n -> p c n", p=128)
                )
            for b in range(B_LOC):
                nc.sync.dma_start(
                    out=gtT_t[b], in_=gtT_d[b].rearrange("(c p) n -> p c n", p=128)
                )
            # gpsimd queue (parallel to sync): fp8 masks, cast to fp16 in-flight
            for b in range(B_LOC):
                for h in range(2):
                    nc.gpsimd.dma_start(
                        out=msb[b][:, 4 * h : 4 * h + 4, :],
                        in_=msC_d[b, 512 * h : 512 * (h + 1), :].rearrange(
                            "(t p) i -> p t i", p=128
                        ),
                    )

            # ---- Phase 1 ----
            # Per-tile interleave of A (sigmoid on ACT, mask-mult on DVE) and
            # C (gconv1 matmuls on PE, relu on DVE) so the DVE queue drains
            # each batch's At and o1t tiles in lockstep with the sigmoids.
            def stage_AC_D0h1(b, ps_h1):
                # Per-jc: sigmoid (ACT), mask-mult (DVE), gconv1 (PE), relu
                # (DVE), then the first 4 it-groups of the big contraction so
                # the PE rides the sigmoid window. The 4 PSUM groups stay open
                # across the whole jc sweep.
                for jc in range(NT):
                    nc.scalar.activation(
                        out=At[b][:, jc, :], in_=lirow_t[b], func=AF.Sigmoid,
                        bias=ljT_t[b][:, jc : jc + 1], scale=1.0,
                    )
                    nc.vector.tensor_mul(
                        At[b][:, jc, :], At[b][:, jc, :], msb[b][:, jc, :]
                    )
                    ps = cpp.tile([128, MID], mybir.dt.float32, tag="cps")
                    nc.tensor.matmul(ps, lhsT=ones_t, rhs=b1_t, start=True, stop=False)
                    for cc in range(2):
                        nc.tensor.matmul(
                            ps[:, 256 * cc : 256 * (cc + 1)],
                            lhsT=xT_t[b][:, cc, jc * 128 : (jc + 1) * 128],
                            rhs=w1h_t[:, cc, :],
                            start=False, stop=(cc == 1),
                            skip_group_check=True,
                        )
                    # relu on DVE so o1t is ready independent of the ACT queue
                    nc.vector.tensor_scalar_max(o1t[b][:, jc, :], ps, 0.0)

            def stage_B(b):
                for nt in range(NT):
                    ps = bpp.tile([128, OUT], mybir.dt.float32, tag="bps", name="bps")
                    nc.tensor.matmul(ps, lhsT=ones_t, rhs=bg_t, start=True, stop=False)
                    for cc in range(2):
                        nc.tensor.matmul(
                            ps,
                            lhsT=gtT_t[b][:, cc, nt * 128 : (nt + 1) * 128],
                            rhs=wg_t[:, cc, :],
                            start=False, stop=(cc == 1),
                        )
                    nc.scalar.activation(
                        out=gts_s[b][:, nt, :], in_=ps, func=AF.Relu
                    )
                nc.sync.dma_start(
                    out=gts_d[b].rearrange("(t p) f -> p t f", p=128), in_=gts_s[b]
                )

            def ln1_apply(b, it, ps):
                    sv = workp.tile([128, 6], f32, tag="sv")
                    nc.vector.bn_stats(out=sv, in_=ps)
                    mv = workp.tile([128, 2], f32, tag="mv")
                    nc.vector.bn_aggr(out=mv, in_=sv)
                    std = workp.tile([128, 1], f32, tag="std")
                    nc.scalar.activation(
                        out=std, in_=mv[:, 1:2], func=AF.Sqrt, bias=eps_t
                    )
                    rstd = workp.tile([128, 1], f32, tag="rstd")
                    nc.vector.reciprocal(out=rstd, in_=std)
                    outer = workp.tile([128, MID], f16, tag="outer")
                    nc.vector.tensor_scalar_mul(outer, g1row_t, rstd)
                    ln = workp.tile([128, MID], f16, tag="ln")
                    nc.vector.scalar_tensor_tensor(
                        out=ln, in0=ps, scalar=mv[:, 0:1], in1=outer,
                        op0=OP.subtract, op1=OP.mult,
                    )
                    if beta1_nz:
                        nc.vector.tensor_add(ln, ln, beta1_t)
                    nc.vector.tensor_add(o1nT[b][:, it, :], ln, o1t[b][:, it, :])

            def stage_D0_rest(b, ps_h1):
                for it in range(4):
                    ln1_apply(b, it, ps_h1[it])
                for it in range(4, NT):
                    ps = mmp.tile([128, MID], mybir.dt.float32, tag="ps")
                    for jc in range(NT):
                        nc.tensor.matmul(
                            ps,
                            lhsT=At[b][:, jc, it * 128 : (it + 1) * 128],
                            rhs=o1t[b][:, jc, :],
                            start=(jc == 0), stop=(jc == NT - 1),
                        )
                    ln1_apply(b, it, ps)

            def stage_D(b):
                for it in range(NT):
                    ps = mmp.tile([128, MID], mybir.dt.float32, tag="ps")
                    for jc in range(NT):
                        nc.tensor.matmul(
                            ps,
                            lhsT=At[b][:, jc, it * 128 : (it + 1) * 128],
                            rhs=o1t[b][:, jc, :],
                            start=(jc == 0), stop=(jc == NT - 1),
                        )
                    ln1_apply(b, it, ps)

            def stage_T(b):
                # DMA xbar transpose, 4 it-tiles at a time: [128, 2048] ->
                # o1n[128, (it,mc), 128] with row = mc*128 + mp.
                for q in range(2):
                    nc.sync.dma_start(
                        out=o1n[b][:, 4 * q : 4 * q + 4, :, :],
                        in_=o1nT[b][:, 4 * q : 4 * q + 4, :],
                        transpose=True,
                    )

            def stage_E(b):
                for jt in range(NT):
                    ps = mmp.tile([128, MID], mybir.dt.float32, tag="ps")
                    p256 = ps[:, :OUT]
                    nc.tensor.matmul(p256, lhsT=ones_t, rhs=b2_t, start=True, stop=False)
                    for g in range(4):
                        nc.tensor.matmul(
                            p256[:, 64 * g : 64 * (g + 1)],
                            lhsT=o1n[b][:, jt, g, :],
                            rhs=w2g_t[:, g, :],
                            start=False, stop=(g == 3),
                            skip_group_check=True,
                        )
                    nc.scalar.activation(out=o2t[b][:, jt, :], in_=p256, func=AF.Relu)

            def stage_F(b):
                for it in range(NT):
                    ps = mmp.tile([128, MID], mybir.dt.float32, tag="ps")
                    p256 = ps[:, :OUT]
                    for jc in range(NT):
                        nc.tensor.matmul(
                            p256,
                            lhsT=At[b][:, jc, it * 128 : (it + 1) * 128],
                            rhs=o2t[b][:, jc, :],
                            start=(jc == 0), stop=(jc == NT - 1),
                        )
                    sv = workp.tile([128, 6], f32, tag="sv")
                    nc.vector.bn_stats(out=sv, in_=p256)
                    mv = workp.tile([128, 2], f32, tag="mv")
                    nc.vector.bn_aggr(out=mv, in_=sv)
                    std = workp.tile([128, 1], f32, tag="std")
                    nc.scalar.activation(
                        out=std, in_=mv[:, 1:2], func=AF.Sqrt, bias=eps_t
                    )
                    rstd = workp.tile([128, 1], f32, tag="rstd")
                    nc.vector.reciprocal(out=rstd, in_=std)
                    outer2 = workp.tile([128, OUT], f16, tag="outer2")
                    nc.vector.tensor_scalar_mul(outer2, g2row_t, rstd)
                    nc.vector.scalar_tensor_tensor(
                        out=node_s[b][:, it, :], in0=p256, scalar=mv[:, 0:1],
                        in1=outer2, op0=OP.subtract, op1=OP.mult,
                    )
                    if beta2_nz:
                        nc.vector.tensor_add(
                            node_s[b][:, it, :], node_s[b][:, it, :], beta2_t
                        )
                    # residual add on gpsimd (idle engine) to shorten the DVE tail
                    nc.gpsimd.tensor_add(
                        out2_s[b][:, it, :], node_s[b][:, it, :], o2t[b][:, it, :]
                    )
                    if it % 2 == 1:
                        h = it // 2
                        nc.sync.dma_start(
                            out=node_d[b].rearrange("(t p) f -> p t f", p=128)[
                                :, 2 * h : 2 * h + 2, :
                            ],
                            in_=node_s[b][:, 2 * h : 2 * h + 2, :],
                        )
                        nc.sync.dma_start(
                            out=out2_d[b].rearrange("(t p) f -> p t f", p=128)[
                                :, 2 * h : 2 * h + 2, :
                            ],
                            in_=out2_s[b][:, 2 * h : 2 * h + 2, :],
                        )

            def stage_sig(b):
                for jt in range(NT):
                    nc.scalar.activation(
                        out=At[b][:, jt, :], in_=lirow_t[b], func=AF.Sigmoid,
                        bias=ljT_t[b][:, jt : jt + 1], scale=1.0,
                    )

            def stage_Atm_gpsimd(b):
                # mask-mult on the gpsimd engine (frees the DVE queue; the
                # sigmoids pace this anyway)
                for jt in range(NT):
                    nc.gpsimd.tensor_mul(
                        At[b][:, jt, :], At[b][:, jt, :], msb[b][:, jt, :]
                    )

            def stage_Conly(b):
                for jt in range(NT):
                    ps = cpp.tile([128, MID], mybir.dt.float32, tag="cps")
                    nc.tensor.matmul(ps, lhsT=ones_t, rhs=b1_t, start=True, stop=False)
                    for cc in range(2):
                        nc.tensor.matmul(
                            ps[:, 256 * cc : 256 * (cc + 1)],
                            lhsT=xT_t[b][:, cc, jt * 128 : (jt + 1) * 128],
                            rhs=w1h_t[:, cc, :],
                            start=False, stop=(cc == 1),
                            skip_group_check=True,
                        )
                    nc.vector.tensor_scalar_max(o1t[b][:, jt, :], ps, 0.0)

            # Emission order tuned against in-order engine queues:
            # PE: C0 D0 C1 B0 B1 D1 E0 F0 E1 F1; ACT: sig0 sig1 then LN ops;
            # DVE: b0 A/C pairs, D0-LN, C1 relus, D1-LN, F-LN chains;
            # gpsimd: mask DMAs, batch-1 At mults, out2 adds.
            stage_AC_D0h1(0, None)
            stage_AC_D0h1(1, None)
            stage_D(0)
            stage_T(0)
            stage_D(1)
            stage_T(1)
            stage_B(0)
            stage_B(1)
            stage_E(0)
            stage_F(0)
            stage_E(1)
            stage_F(1)

    nc.compile()
    return nc


def _compute_col_fast(m1, m2, sm):
    """Exact col == ones proof via a cheap sufficient condition, else None."""
    if m1.min() < 0.0 or m2.min() < 0.0 or sm.min() < 0.0:
        return None
    spos = (sm > 0).astype(F32)
    colnz = np.zeros(N, dtype=bool)
    nz1max = 0.0
    nz2max = 0.0
    for b in range(B):
        p1 = (m1[b] > 0).astype(F32)
        p2 = (m2[b] > 0).astype(F32)
        nz1max = max(nz1max, float((p1 @ spos[b]).max()))
        nz2max = max(nz2max, float((p2 @ spos[b]).max()))
        colnz |= ((p1 + p2).max(axis=0) > 0) & (spos[b] > 0)
    if nz1max <= CHILDS // 4 and nz2max <= CHILDS // 2 and colnz.all():
        return np.ones(N, dtype=F32)
    return None


def _compute_col_slow(m1, m2, sm, li, lj):
    """Exact replica of the reference top-k column-union (numpy)."""
    k4, k2 = CHILDS // 4, CHILDS // 2
    col = np.zeros(N, dtype=bool)
    for b in range(B):
        logits = li[b][:, None] + lj[b][None, :]
        a = 1.0 / (1.0 + np.exp(-logits.astype(F32)))
        mr1 = m1[b] * sm[b][None, :]
        mr2 = m2[b] * sm[b][None, :]
        a1 = a * mr1
        a2 = a * mr2
        # lax.top_k ties -> lowest index; stable argsort on (-a) reproduces it.
        col[np.argsort(-a1, axis=1, kind="stable")[:, :k4].ravel()] = True
        col[np.argsort(a1, axis=1, kind="stable")[:, :k4].ravel()] = True
        col[np.argsort(-a2, axis=1, kind="stable")[:, :k2].ravel()] = True
        col[np.argsort(a2, axis=1, kind="stable")[:, :k4].ravel()] = True
    return col.astype(F32)


def kernel(**inputs):
    x = np.ascontiguousarray(np.asarray(inputs["x"], dtype=F32))
    m1 = np.asarray(inputs["masks_roi1"], dtype=F32)
    m2 = np.asarray(inputs["masks_roi2"], dtype=F32)
    sm = np.asarray(inputs["score_mask"], dtype=F32)
    gt = np.asarray(inputs["gt_feat"], dtype=F32)
    W_att = np.asarray(inputs["W_att"], dtype=F32)
    b_att = np.asarray(inputs["b_att"], dtype=F32)
    W1 = np.asarray(inputs["W1"], dtype=F32)
    b1 = np.asarray(inputs["b1"], dtype=F32)
    W2 = np.asarray(inputs["W2"], dtype=F32)
    b2 = np.asarray(inputs["b2"], dtype=F32)
    g1 = np.asarray(inputs["g1"], dtype=F32)
    beta1 = np.asarray(inputs["beta1"], dtype=F32)
    g2 = np.asarray(inputs["g2"], dtype=F32)
    beta2 = np.asarray(inputs["beta2"], dtype=F32)
    Wg = np.asarray(inputs["Wg"], dtype=F32)
    bg = np.asarray(inputs["bg"], dtype=F32)

    assert x.shape == (B, N, CIN) and W_att.shape == (2 * CIN, 1)

    # ---- host prep: tiny vector math + layout/dtype staging ----
    lj = x.reshape(B * N, CIN) @ W_att[:CIN, 0]
    lj = lj.reshape(B, N)
    li = x.reshape(B * N, CIN) @ W_att[CIN:, 0]
    li = li.reshape(B, N) + b_att[0]

    col = _compute_col_fast(m1, m2, sm)
    if col is None:
        col = _compute_col_slow(m1, m2, sm, li, lj)

    # Combined transposed mask: msC[b,j,i] = (m1[b,i,j]+m2[b,i,j])*sm[b,j]*col[j]
    # with the f_diag term folded into the diagonal as f/sigmoid(li+lj) so the
    # device-side multiply by sigmoid reconstructs it.  Mask values {0,1,2} are
    # exact in fp8; the diagonal entries round at ~6% relative, which after the
    # (1/CHILDS-scaled) contraction is noise well below the fp16 level.
    smcol = sm * col[None, :]
    msC = (m1.transpose(0, 2, 1) + m2.transpose(0, 2, 1)) * smcol[:, :, None]
    f = (sm == 0).astype(F32)
    diag_sig = 1.0 / (1.0 + np.exp(-(li + lj).astype(np.float64)))
    idx = np.arange(N)
    msC[:, idx, idx] += f / diag_sig.astype(F32)
    msC = msC.astype(F8)

    xT = np.ascontiguousarray(x.transpose(0, 2, 1)).astype(F16)
    gtT = np.ascontiguousarray(gt.transpose(0, 2, 1)).astype(F16)
    lirow = np.broadcast_to(li.astype(F16)[:, None, :], (B, 128, N)).copy()
    ljT = np.ascontiguousarray(lj.reshape(B, NT, 128).transpose(0, 2, 1)).astype(F32)

    # Weights: block-diagonal transposed layouts for the grouped convs.
    w1bd = np.zeros((CIN, MID), dtype=F32)
    for g in range(G):
        w1bd[64 * g : 64 * (g + 1), 128 * g : 128 * (g + 1)] = W1[
            128 * g : 128 * (g + 1), :
        ].T
    w1h = np.stack(
        [w1bd[0:128, 0:256], w1bd[128:256, 256:512]]
    ).astype(F16)  # [2, 128, 256]
    w2g = np.stack(
        [W2[64 * g : 64 * (g + 1), :].T for g in range(G)]
    ).astype(F16)  # [4, 128, 64]
    wgK = np.ascontiguousarray(Wg.reshape(2, 128, OUT)).astype(F16)

    shared = {
        "wgK": wgK,
        "w1h": w1h,
        "w2g": w2g,
        "bgrow": bg.reshape(1, OUT).astype(F16),
        "b1row": b1.reshape(1, MID).astype(F16),
        "b2row": b2.reshape(1, OUT).astype(F16),
        "g1row": np.broadcast_to(g1.astype(F16)[None, :], (128, MID)).copy(),
        "g2row": np.broadcast_to(g2.astype(F16)[None, :], (128, OUT)).copy(),
        "beta1row": beta1.reshape(1, MID).astype(F32),
        "beta2row": beta2.reshape(1, OUT).astype(F32),
        "onescol": np.ones((1, 128), dtype=F16),
    }
    in_maps = []
    for c in range(NCORES):
        s = slice(B_LOC * c, B_LOC * (c + 1))
        in_maps.append(
            {
                "msC": msC[s],
                "xT": xT[s],
                "gtT": gtT[s],
                "lirow": lirow[s],
                "ljT": ljT[s],
                **shared,
            }
        )

    beta_key = (bool(np.any(beta1)), bool(np.any(beta2)))
    if beta_key not in _PROGRAM_CACHE:
        _PROGRAM_CACHE[beta_key] = _build_program(*beta_key)
    nc = _PROGRAM_CACHE[beta_key]

    global _LAST_IN_MAPS
    _LAST_IN_MAPS = in_maps

    from concourse.bass_utils import run_bass_kernel_spmd

    res = run_bass_kernel_spmd(nc, in_maps, core_ids=list(range(NCORES)))
    results = res.results if hasattr(res, "results") else res

    output2 = np.concatenate([results[c]["out2"] for c in range(NCORES)], axis=0)
    gts = np.concatenate([results[c]["gts"] for c in range(NCORES)], axis=0)
    node_feat = np.concatenate([results[c]["node"] for c in range(NCORES)], axis=0)
    return output2.astype(F32), gts.astype(F32), node_feat.astype(F32)
